# revision 15
# baseline (speedup 1.0000x reference)
"""Trainium2 Bass kernel for nn_ExtractModel (retrieval_knn).

Strategy (vocab-sharded across 8 NeuronCores, per the sharding hint):
  - every core computes the normalized word/unit embeddings (small matmuls)
  - the vocab V=10000 is split round-robin-within-length-group across cores so
    all cores share one instruction stream (groups padded to a common size
    with duplicates of the group's first word -- exact-duplicate scores keep
    first-index argmax semantics intact)
  - per core: acc[bl, w] = sum_{j<k_w} wn[b, l+j, :] . un[seg[w, j], :]
    computed as PE matmuls: lhsT = wnT[:, (b, l+j)] slices (zero-padded past
    l=63, matching reference's sim_pad), rhs = unT columns gathered by GPSIMD
    ap_gather.  Words sorted by length ascending => step j's active words are
    a contiguous suffix; accumulate over (j, d-half) in PSUM (fp32).
  - per length-group: DVE max8 + max_index give per-(b,l) max + argmax
  - host: viability mask, 8-shard combine, exact reference final semantics.

Self-contained: hardcodes shapes; reads nothing from /root/problem.
"""
import os
import numpy as np

MIN_LEN, MAX_LEN, THRESH, NEG = 4, 10, 0.05, -1e9
B, L, F, D, U, V = 16, 64, 60, 256, 600, 10000
J = MAX_LEN
E = MAX_LEN - MIN_LEN + 1          # 7
NCORES = 8
LPAD = L + J                       # 74
BL = B * L                         # 1024
NT = BL // 128                     # 8 bl-tiles
UPAD = 640                         # unit dim padded to 5*128
DH = D // 128                      # 2 contraction halves

# perf knobs (env-overridable for experiments)
MM_F32R = os.environ.get("KERNEL_MM_F32R", "1") == "1"
RUN_SIM = os.environ.get("KERNEL_SIM", "0") == "1"
TRACE = os.environ.get("KERNEL_TRACE", "0") == "1"
LAST_RESULTS = None  # BassKernelResults from the most recent kernel() call


# ----------------------------------------------------------------------------
# host-side prep
# ----------------------------------------------------------------------------

def build_shards(indexed_segments, vocab_length):
    """Per-core global-id tables; all cores share padded group sizes."""
    vocab_length = np.asarray(vocab_length)
    group_sizes = []
    per_core_groups = [[] for _ in range(NCORES)]
    for k in range(MIN_LEN, MAX_LEN + 1):
        ids = np.nonzero(vocab_length == k)[0]
        per_core = [ids[c::NCORES] for c in range(NCORES)]
        gsz = max(len(p) for p in per_core) if len(ids) else 0
        if gsz:
            gsz = max(gsz, 8)      # vector.max needs free size >= 8
        group_sizes.append(gsz)
        for c in range(NCORES):
            lst = list(per_core[c])
            if gsz and not lst:
                lst = [int(ids[0])]
            while len(lst) < gsz:
                lst.append(lst[0])
            per_core_groups[c].append(lst)
    shard_word = [
        np.array([w for g in per_core_groups[c] for w in g], dtype=np.int64)
        for c in range(NCORES)
    ]
    Vs = int(sum(group_sizes))
    return shard_word, group_sizes, Vs


def build_gather_idx(shard_word_c, group_sizes, indexed_segments):
    """j-major gather index list with suffix structure."""
    ks = np.repeat(np.arange(MIN_LEN, MAX_LEN + 1), group_sizes)
    Vs = len(ks)
    idx_list, offs, starts = [], [], []
    for j in range(J):
        S_j = int(np.searchsorted(ks, j + 1, side="left"))
        offs.append(len(idx_list))
        starts.append(S_j)
        words = shard_word_c[S_j:]
        idx_list.extend(np.asarray(indexed_segments)[words, j].tolist())
    return np.array(idx_list, dtype=np.int16), offs, starts


def wrap_idx(idx_list, kidx_pad):
    """ap_gather idx layout: tile[p, s] = idx[16*s + p], replicated to 128p."""
    idx = np.zeros(kidx_pad, np.int16)
    idx[: len(idx_list)] = idx_list
    t16 = idx.reshape(-1, 16).T            # [16, kidx_pad/16]
    return np.tile(t16, (8, 1)).astype(np.int16)  # [128, Sidx]


# ----------------------------------------------------------------------------
# bass program
# ----------------------------------------------------------------------------

def build_nc(group_sizes, offs, starts, Vs, kidx_pad):
    import concourse.bass as bass
    import concourse.bacc as bacc
    import concourse.mybir as mybir
    import concourse.tile as tile
    from concourse import library_config

    fp32 = mybir.dt.float32
    sidx = kidx_pad // 16
    vs_pad = 512 * ((Vs + 511) // 512)          # psum chunking
    assert vs_pad <= 1536, f"Vs={Vs} needs more than 3 psum banks"
    chunks = [(c, min(c + 512, Vs)) for c in range(0, Vs, 512)]
    # last j (step) that touches each chunk: max j with starts[j] < c_hi
    last_j = [max(j for j in range(J) if starts[j] < hi) for (_, hi) in chunks]

    # Bacc (not raw Bass): its compile() pass splits multi-waits into event
    # semaphores (TRN2 allows 1 wait/instruction), auto-inserts GPSIMD library
    # loads, and runs extended-inst ISA codegen.
    nc = bacc.Bacc(None, target_bir_lowering=False)
    x_d = nc.declare_dram_parameter("x", [BL, F], fp32, isOutput=False)
    uf_d = nc.declare_dram_parameter("uf", [UPAD, F], fp32, isOutput=False)
    w_d = nc.declare_dram_parameter("w", [F, D], fp32, isOutput=False)
    id_d = nc.declare_dram_parameter("ident", [128, 128], fp32, isOutput=False)
    gi_d = nc.declare_dram_parameter("gidx", [128, sidx], mybir.dt.int16, isOutput=False)
    rk_d = nc.declare_dram_parameter("recipk", [128, E], fp32, isOutput=False)
    ov_d = nc.declare_dram_parameter("oval", [NT, 128, E], fp32, isOutput=True)
    oi_d = nc.declare_dram_parameter("oidx", [NT, 128, E], mybir.dt.uint32, isOutput=True)

    Copy = mybir.ActivationFunctionType.Copy
    Square = mybir.ActivationFunctionType.Square

    with tile.TileContext(nc) as tc:
        with (
            tc.tile_pool(name="consts", bufs=1) as consts,
            tc.tile_pool(name="scratch", bufs=3) as scratch,
            tc.tile_pool(name="outsb", bufs=3) as outsb,
            tc.tile_pool(name="ph1psum", bufs=2, space="PSUM") as ph1,
            tc.tile_pool(name="accpsum", bufs=2, space="PSUM") as accp,
        ):
            ident = consts.tile([128, 128], fp32, tag="ident")
            nc.sync.dma_start(out=ident, in_=id_d[:, :])
            w_sb = consts.tile([F, D], fp32, tag="w")
            nc.sync.dma_start(out=w_sb, in_=w_d[:, :])
            gidx = consts.tile([128, sidx], mybir.dt.int16, tag="gidx")
            nc.sync.dma_start(out=gidx, in_=gi_d[:, :])
            recip = consts.tile([128, E], fp32, tag="recip")
            nc.sync.dma_start(out=recip, in_=rk_d[:, :])

            xT = consts.tile([F, NT, 128], fp32, tag="xT")
            ufT = consts.tile([F, 5, 128], fp32, tag="ufT")
            wn_all = consts.tile([128, NT, D], fp32, tag="wn")
            un_all = consts.tile([128, 5, D], fp32, tag="un")
            # per bl-tile, l-major with the tile's two b's interleaved so the
            # j-shifted 128-column weight slice is contiguous (matmul lhsT
            # APs must flatten to one free dim).  partition p of the acc psum
            # then maps to (l = p//2, b = 2t + p%2).
            wnT = consts.tile([128, DH, NT, LPAD, 2], fp32, tag="wnT")
            unT = consts.tile([128, DH, UPAD], fp32, tag="unT")
            gath = consts.tile([128, DH, kidx_pad], fp32, tag="gath")

            nc.vector.memset(wnT, 0.0)

            # ---- phase A: transpose x and uf ----
            for t in range(NT):
                xt = scratch.tile([128, F], fp32, tag="ld")
                nc.sync.dma_start(out=xt, in_=x_d[t * 128:(t + 1) * 128, :])
                ps = ph1.tile([128, 256], fp32, tag="ph1")
                nc.tensor.transpose(ps[:F, :128], xt, ident)
                nc.vector.tensor_copy(out=xT[:, t, :], in_=ps[:F, :128])
            for c in range(5):
                ut = scratch.tile([128, F], fp32, tag="ld")
                nc.sync.dma_start(out=ut, in_=uf_d[c * 128:(c + 1) * 128, :])
                ps = ph1.tile([128, 256], fp32, tag="ph1")
                nc.tensor.transpose(ps[:F, :128], ut, ident)
                nc.vector.tensor_copy(out=ufT[:, c, :], in_=ps[:F, :128])

            # ---- phase B: embed + normalize ----
            def embed_norm(lhsT, out_sl):
                ps = ph1.tile([128, 256], fp32, tag="ph1")
                nc.tensor.matmul(ps, lhsT, w_sb, start=True, stop=True)
                sq = scratch.tile([128, D], fp32, tag="sq")
                ss = scratch.tile([128, 1], fp32, tag="ss")
                nc.scalar.activation(out=sq, in_=ps, func=Square, accum_out=ss)
                nrm = scratch.tile([128, 1], fp32, tag="nrm")
                nc.scalar.sqrt(nrm, ss)
                nc.vector.tensor_scalar_add(nrm, nrm, 1e-8)
                rn = scratch.tile([128, 1], fp32, tag="rn")
                nc.vector.reciprocal(rn, nrm)
                nc.vector.tensor_scalar_mul(out_sl, ps, rn)

            for t in range(NT):
                embed_norm(xT[:, t, :], wn_all[:, t, :])
            for c in range(5):
                embed_norm(ufT[:, c, :], un_all[:, c, :])

            # ---- phase C: transpose to K-major layouts ----
            for t in range(NT):
                for dh in range(DH):
                    ps = ph1.tile([128, 256], fp32, tag="ph1")
                    nc.tensor.transpose(
                        ps[:, :128], wn_all[:, t, dh * 128:(dh + 1) * 128], ident)
                    dst = wnT[:, dh, t, 0:64, :]
                    nc.vector.tensor_copy(
                        out=dst, in_=ps[:, :128].rearrange("p (b l) -> p l b", b=2))
            for c in range(5):
                for dh in range(DH):
                    ps = ph1.tile([128, 256], fp32, tag="ph1")
                    nc.tensor.transpose(
                        ps[:, :128], un_all[:, c, dh * 128:(dh + 1) * 128], ident)
                    nc.vector.tensor_copy(
                        out=unT[:, dh, c * 128:(c + 1) * 128], in_=ps[:, :128])

            # ---- phase D: gather unT columns for every (word, j) ----
            nc.gpsimd.load_library(library_config.ap_gather)
            for dh in range(DH):
                nc.gpsimd.ap_gather(
                    out_ap=gath[:, dh, :],
                    in_ap=unT[:, dh, :],
                    idxs_ap=gidx,
                    channels=128,
                    num_elems=UPAD,
                    d=1,
                    num_idxs=kidx_pad,
                )

            # ---- phase E: accumulate + reduce per bl-tile ----
            mm_dt = mybir.dt.float32r if MM_F32R else fp32
            for t in range(NT):
                acc = accp.tile([128, vs_pad], fp32, tag="acc")
                for j in range(J):
                    S_j, off_j = starts[j], offs[j]
                    lhs = wnT[:, :, t, j:j + 64, :]
                    for dh in range(DH):
                        for ci, (lo, hi) in enumerate(chunks):
                            clo, chi = max(lo, S_j), hi
                            if clo >= chi:
                                continue
                            rhs = gath[:, dh, off_j + (clo - S_j): off_j + (chi - S_j)]
                            nc.tensor.matmul(
                                acc[:, clo:chi],
                                lhs[:, dh].bitcast(mm_dt),
                                rhs.bitcast(mm_dt),
                                start=(j == 0 and dh == 0),
                                stop=(j == last_j[ci] and dh == DH - 1),
                            )
                accS = scratch.tile([128, Vs], fp32, tag="accS")
                nc.scalar.activation(out=accS, in_=acc[:, :Vs], func=Copy)
                valst = outsb.tile([128, E, 8], fp32, tag="valst")
                idxst = outsb.tile([128, E, 8], mybir.dt.uint32, tag="idxst")
                g_lo = 0
                for e, gsz in enumerate(group_sizes):
                    if gsz == 0:
                        continue
                    sl = accS[:, g_lo:g_lo + gsz]
                    nc.vector.max(out=valst[:, e, :], in_=sl)
                    nc.vector.max_index(
                        out=idxst[:, e, :], in_max=valst[:, e, :], in_values=sl)
                    g_lo += gsz
                valc = outsb.tile([128, E], fp32, tag="valc")
                idxc = outsb.tile([128, E], mybir.dt.uint32, tag="idxc")
                nc.vector.tensor_mul(valc, valst[:, :, 0], recip)
                nc.vector.tensor_copy(out=idxc, in_=idxst[:, :, 0])
                nc.sync.dma_start(out=ov_d[t], in_=valc)
                nc.sync.dma_start(out=oi_d[t], in_=idxc)
    nc.compile()
    return nc


# ----------------------------------------------------------------------------
# host-side combine (exact reference final semantics)
# ----------------------------------------------------------------------------

def dev_to_bl(a):
    """Device [NT, 128, E] (partition p = (l, b2) interleaved) -> [BL, E]."""
    return np.ascontiguousarray(
        a.reshape(NT, L, 2, E).transpose(0, 2, 1, 3)).reshape(BL, E)

def host_combine(vals, gvoc, lengths):
    """vals/gvoc: [NCORES, B, L, E] fp32 / int64 (global vocab ids)."""
    lengths = np.asarray(lengths)
    k = np.arange(E) + MIN_LEN
    ends = np.arange(L)[:, None] + k[None, :] - 1
    viable = ends[None] < lengths[:, None, None]            # [B, L, E]
    vmax = vals.max(axis=0)                                  # [B, L, E]
    tie = vals == vmax[None]
    matched_vocab = np.where(tie, gvoc, V + 1).min(axis=0)   # min global id on tie
    value = np.where(viable, vmax, np.float32(NEG))
    matched_vocab = np.where(viable, matched_vocab, 0)
    flat = value.reshape(B, -1)
    best_inds = flat.argmax(-1)
    best_scores = flat.max(-1).astype(np.float32)
    best_starts = (best_inds // E).astype(np.int32)
    best_ends = (best_inds % E + best_starts + MIN_LEN - 1).astype(np.int32)
    any_matched = (value > THRESH).reshape(B, -1).any(-1)
    best_vocab = matched_vocab.reshape(B, -1)[np.arange(B), best_inds].astype(np.int32)
    return best_scores, best_starts, best_ends, any_matched, best_vocab


# ----------------------------------------------------------------------------
# entry point
# ----------------------------------------------------------------------------

def kernel(x, unit_feats, W, lengths, indexed_segments, vocab_length):
    x = np.asarray(x, np.float32).reshape(BL, F)
    uf = np.zeros((UPAD, F), np.float32)
    uf[:U] = np.asarray(unit_feats, np.float32)
    W = np.asarray(W, np.float32)
    seg = np.asarray(indexed_segments)
    vlen = np.asarray(vocab_length)

    shard_word, group_sizes, Vs = build_shards(seg, vlen)
    idx0, offs, starts = build_gather_idx(shard_word[0], group_sizes, seg)
    kidx = len(idx0)
    kidx_pad = 32 * ((kidx + 31) // 32)

    ident = np.eye(128, dtype=np.float32)
    recipk = np.tile(
        (1.0 / (np.arange(E) + MIN_LEN)).astype(np.float32), (128, 1))

    in_maps = []
    for c in range(NCORES):
        idx_c, offs_c, starts_c = build_gather_idx(shard_word[c], group_sizes, seg)
        assert offs_c == offs and starts_c == starts and len(idx_c) == kidx
        in_maps.append({
            "x": x, "uf": uf, "w": W, "ident": ident,
            "gidx": wrap_idx(idx_c, kidx_pad), "recipk": recipk,
        })

    nc = build_nc(group_sizes, offs, starts, Vs, kidx_pad)

    if RUN_SIM:
        from concourse.bass_interp import CoreSim
        results = []
        for c in range(NCORES):
            sim = CoreSim(nc)
            for k_, v_ in in_maps[c].items():
                sim.tensor(k_)[:] = v_
            sim.simulate()
            results.append({"oval": np.array(sim.tensor("oval")),
                            "oidx": np.array(sim.tensor("oidx"))})
    else:
        from concourse.bass_utils import run_bass_kernel_spmd
        global LAST_RESULTS
        LAST_RESULTS = run_bass_kernel_spmd(
            nc, in_maps, core_ids=list(range(NCORES)), trace=TRACE)
        results = LAST_RESULTS.results

    vals = np.zeros((NCORES, B, L, E), np.float32)
    gvoc = np.zeros((NCORES, B, L, E), np.int64)
    g_lo_tbl = np.concatenate([[0], np.cumsum(group_sizes)])[:E]
    for c in range(NCORES):
        ov = dev_to_bl(np.asarray(results[c]["oval"]))
        oi = dev_to_bl(np.asarray(results[c]["oidx"]).astype(np.int64))
        for e, gsz in enumerate(group_sizes):
            if gsz == 0:
                ov[:, e] = NEG
                oi[:, e] = 0
            else:
                oi[:, e] = np.minimum(oi[:, e], gsz - 1) + g_lo_tbl[e]
        vals[c] = ov.reshape(B, L, E)
        gvoc[c] = shard_word[c][oi].reshape(B, L, E)
        # empty groups: point at vocab 0 with NEG score; combine ignores them
        for e, gsz in enumerate(group_sizes):
            if gsz == 0:
                gvoc[c, :, :, e] = 0

    return host_combine(vals, gvoc, np.asarray(lengths))
